# revision 22
# baseline (speedup 1.0000x reference)
"""EdgeConv GNN (4 layers) on 8 Trainium2 NeuronCores.

Algebraic restructure: with y = x @ theta_w.T and
v = x @ (phi_w - theta_w).T + (phi_b + theta_b),
    msg_e = theta(x[src]-x[dst]) + theta_b + phi(x[dst]) + phi_b
          = y[src] + v[dst]
and since v[dst] is constant within a dst segment:
    out = relu(v + segment_max(y[src], dst))
(nodes with no in-edges come out of segment_max at -1e30 -> relu -> 0,
matching the reference's where(isneginf, 0) + relu).

Distribution: nodes sharded by dst across 8 cores (graph parallel).
Each layer: per-core bf16 matmuls produce its y-shard -> AllGather the
bf16 y table -> SWDGE dma_gather of y rows by src in dst-sorted slot
order -> per-block max-tree reduce on DVE (contiguous tensor_tensor,
not strided tensor_reduce) -> + v -> relu.

dma_gather indices are int16 (<= 32767) so the 50176-row table is
addressed through two windows: A = rows [0, 31360) (src cores 0-4) and
B = rows [18816, 50176) (src cores 3-7). Src cores 3-4 edges are flex
(either window); a per-block joint optimization assigns flex edges to
minimize sum_b(KA[b] + KB[b]) where K* are per-128-node-block degree
caps (shared across the 8 cores: single SPMD instruction stream).
"""

import numpy as np
import ml_dtypes

BF16 = ml_dtypes.bfloat16

N = 50000
NCORES = 8
NPC = 6250            # real nodes per core
NPCP = 6272           # padded nodes per core (49 * 128)
F = 128
NL = 4
NB = NPCP // 128      # 49 blocks per core
NTAB = NCORES * NPCP  # 50176 table rows
BASE_B = 3 * NPCP     # 18816: window B base row (cores 3-7)
N_PHANTOM = NPCP - NPC
GMAX = 40             # max chunks per gather call (per window)
NEG = -1.0e30
SKIP = set()          # debug: subset of {"gather", "reduce", "ag", "mm"}

_cache = {}


# ----------------------------------------------------------------------------
# host-side graph preprocessing
# ----------------------------------------------------------------------------

def _prep_graph(src, dst):
    src = np.asarray(src).astype(np.int64)
    dst = np.asarray(dst).astype(np.int64)
    s_core = src // NPC
    d_core = dst // NPC
    core = np.arange(N) // NPC

    fixedA = s_core <= 2          # rows < 18816: window A only
    fixedB = s_core >= 5          # rows >= 31360: window B only
    flexm = ~fixedA & ~fixedB     # src cores 3-4: rows in both windows
    fA = np.bincount(dst[fixedA], minlength=N)
    fB = np.bincount(dst[fixedB], minlength=N)
    fx = np.bincount(dst[flexm], minlength=N)

    # iterate: (node order by balanced degrees) <-> (per-block joint split)
    kA = np.clip((fB - fA + fx + 1) // 2, 0, fx)
    best = None
    for _ in range(3):
        dA, dB = fA + kA, fB + fx - kA
        pos = np.empty(N, np.int64)
        for c in range(NCORES):
            ids = np.arange(c * NPC, (c + 1) * NPC)
            order = np.lexsort((dA[ids] + dB[ids], np.maximum(dA[ids], dB[ids])))
            pos[ids[order]] = N_PHANTOM + np.arange(NPC)
        blk = pos // 128
        KA = np.zeros(NB, np.int64)
        KB = np.zeros(NB, np.int64)
        kA_new = np.zeros(N, np.int64)
        for b in range(NB):
            m = blk == b
            a, x, bb = fA[m], fx[m], fB[m]
            lo, hi = int(a.max()), int((a + x).max())
            bt = None
            for tA in range(lo, hi + 1):
                kab = np.minimum(x, tA - a)
                tB = int((bb + x - kab).max())
                if bt is None or tA + tB < bt[0] + bt[1]:
                    bt = (tA, tB)
            tA, tB = bt
            kmin = np.maximum(0, x - (tB - bb))
            kmax = np.minimum(x, tA - a)
            kA_new[m] = (kmin + kmax) // 2
            KA[b], KB[b] = tA, tB
        tot = int(KA.sum() + KB.sum())
        if best is None or tot < best[0]:
            best = (tot, KA.copy(), KB.copy(), pos.copy(), kA_new.copy())
        kA = kA_new
    _, KA, KB, pos, kA = best

    sig = core * NPCP + pos       # orig node -> table row
    blk = pos // 128
    lane = pos % 128
    cbA = np.r_[0, np.cumsum(KA)]
    cbB = np.r_[0, np.cumsum(KB)]
    CA, CB = int(cbA[-1]), int(cbB[-1])
    assert KA.max() <= GMAX and KB.max() <= GMAX, (KA.max(), KB.max())

    # edge side: fixed by src core; flex edges ranked within their dst group
    sideA = fixedA.copy()
    fe = np.flatnonzero(flexm)
    fe = fe[np.argsort(dst[fe], kind="stable")]
    dsf = dst[fe]
    starts = np.r_[0, np.flatnonzero(np.diff(dsf)) + 1]
    runlen = np.diff(np.r_[starts, len(dsf)])
    rank = np.arange(len(dsf)) - np.repeat(starts, runlen)
    sideA[fe[rank < kA[dsf]]] = True

    # slot arrays (per core); dummy slots use idx 0 (phantom/NEG row in
    # both windows: row 0 and row BASE_B = core 3 pos 0 are phantoms)
    idxA = np.zeros((NCORES, CA * 128), np.int16)
    idxB = np.zeros((NCORES, CB * 128), np.int16)
    for side, idx_arr, cb, base, cap in (
        (True, idxA, cbA, 0, KA), (False, idxB, cbB, BASE_B, KB)
    ):
        e = np.flatnonzero(sideA == side)
        e = e[np.argsort(dst[e], kind="stable")]
        de = dst[e]
        starts = np.r_[0, np.flatnonzero(np.diff(de)) + 1]
        runlen = np.diff(np.r_[starts, len(de)])
        rank = np.arange(len(de)) - np.repeat(starts, runlen)
        assert (rank < cap[blk[de]]).all()
        slot = (cb[blk[de]] + rank) * 128 + lane[de]
        val = sig[src[e]] - base
        assert val.min() >= 0 and val.max() < 32768, (val.min(), val.max())
        idx_arr[d_core[e], slot] = val.astype(np.int16)

    # wrap indices: [n] -> [128, n//16] int16, replicated across 8 groups of 16
    def wrap(a):
        n = a.shape[1]
        w = a.reshape(NCORES, n // 16, 16).transpose(0, 2, 1)  # [c, 16, n/16]
        return np.ascontiguousarray(
            np.broadcast_to(w[:, None, :, :], (NCORES, 8, 16, n // 16))
        ).reshape(NCORES, 128, n // 16)

    # gather groups: consecutive blocks, chunk budget GMAX per window
    groups = []
    b0 = 0
    while b0 < NB:
        nb = 1
        while (
            b0 + nb < NB
            and cbA[b0 + nb + 1] - cbA[b0] <= GMAX
            and cbB[b0 + nb + 1] - cbB[b0] <= GMAX
        ):
            nb += 1
        groups.append((b0, nb, int(cbA[b0]), int(cbA[b0 + nb] - cbA[b0]),
                       int(cbB[b0]), int(cbB[b0 + nb] - cbB[b0])))
        b0 += nb

    return dict(
        sig=sig, pos=pos, KA=KA, KB=KB, cbA=cbA, cbB=cbB, CA=CA, CB=CB,
        idxA=wrap(idxA), idxB=wrap(idxB), groups=groups,
        idxA_flat=idxA, idxB_flat=idxB,
    )


def _prep_weights(theta_w, theta_b, phi_w, phi_b):
    theta_w = np.asarray(theta_w, np.float32)
    phi_w = np.asarray(phi_w, np.float32)
    cb = (np.asarray(theta_b, np.float32) + np.asarray(phi_b, np.float32))
    wcat = np.concatenate(
        [theta_w.transpose(0, 2, 1), (phi_w - theta_w).transpose(0, 2, 1)], axis=2
    )  # [NL, 128(in), 256(out: y|v)]
    # rank-1 bias row: [NL, 1, 256] = [zeros(128) | cb] — accumulated into the
    # yv PSUM tile via a K=1 matmul with a ones column (frees DVE in mm phase)
    cb2 = np.concatenate([np.zeros((NL, 1, F), np.float32), cb[:, None, :]], axis=2)
    return (np.ascontiguousarray(wcat.astype(BF16)),
            np.ascontiguousarray(cb2.astype(BF16)),
            np.ascontiguousarray(cb))


# ----------------------------------------------------------------------------
# device kernel
# ----------------------------------------------------------------------------

def _build_kernel(g):
    import concourse.bacc as bacc
    import concourse.mybir as mybir
    import concourse.tile as tile
    from concourse.masks import make_identity

    KA, KB, groups = g["KA"], g["KB"], g["groups"]
    cbA, cbB = g["cbA"], g["cbB"]
    CA, CB = g["CA"], g["CB"]

    nc = bacc.Bacc("TRN2", target_bir_lowering=False, debug=False,
                   num_devices=NCORES, num_swdge_queues=4)

    bf16 = mybir.dt.bfloat16
    fp32 = mybir.dt.float32
    Alu = mybir.AluOpType
    Act = mybir.ActivationFunctionType

    xin = nc.dram_tensor("xin", [NPCP, F], bf16, kind="ExternalInput")
    idxA_in = nc.dram_tensor("idxA", [128, CA * 8], mybir.dt.int16, kind="ExternalInput")
    idxB_in = nc.dram_tensor("idxB", [128, CB * 8], mybir.dt.int16, kind="ExternalInput")
    wcat_in = nc.dram_tensor("wcat", [NL, F, 2 * F], bf16, kind="ExternalInput")
    cb2_in = nc.dram_tensor("cb2", [NL, 1, 2 * F], bf16, kind="ExternalInput")
    xout = nc.dram_tensor("xout", [NPCP, F], fp32, kind="ExternalOutput")

    with tile.TileContext(nc) as tc:
        with (
            tc.tile_pool(name="const", bufs=1) as constp,
            tc.tile_pool(name="xp", bufs=2) as xp,
            tc.tile_pool(name="vp", bufs=2) as vp,
            tc.tile_pool(name="wp", bufs=2) as wp,
            tc.tile_pool(name="yp", bufs=3) as yp,
            tc.tile_pool(name="xtp", bufs=3) as xtp,
            tc.tile_pool(name="ga", bufs=3) as gap,
            tc.tile_pool(name="gb", bufs=3) as gbp,
            tc.tile_pool(name="tp", bufs=4) as tp,
            tc.tile_pool(name="ps", bufs=4, space="PSUM") as ps,
            tc.tile_pool(name="dram", bufs=2, space="DRAM") as dram,
        ):
            ident = constp.tile([128, 128], bf16)
            make_identity(nc, ident[:])
            ones = constp.tile([1, 128], bf16)
            nc.vector.memset(ones[:], 1.0)
            zeros = constp.tile([128, F], fp32)
            nc.vector.memset(zeros[:], 0.0)
            qn = [0]  # SWDGE queue round-robin across gather calls
            idxA = constp.tile([128, CA * 8], mybir.dt.int16)
            idxB = constp.tile([128, CB * 8], mybir.dt.int16)
            nc.sync.dma_start(idxA[:], idxA_in[:])
            nc.sync.dma_start(idxB[:], idxB_in[:])

            x = xp.tile([128, NB, F], bf16, tag="x")
            nc.sync.dma_start(x[:], xin.rearrange("(b p) f -> p b f", p=128))

            for l in range(NL):
                W = wp.tile([128, 2 * F], bf16, tag="w")
                nc.sync.dma_start(W[:], wcat_in[l])
                cb2 = wp.tile([1, 2 * F], bf16, tag="cb2")
                nc.sync.dma_start(cb2[:], cb2_in[l])

                y_ag_in = dram.tile([NPCP, F], bf16, tag="yag")
                y_all = dram.tile([NTAB, F], bf16, tag="yall", addr_space="Shared")

                v = vp.tile([128, NB, F], bf16, tag="v")

                # ---- matmul phase: y (table, bf16) and v; DVE-free so the
                # vector engine stays on the (previous layer's) reduce work.
                # Descending block order matches the gather phase's descending
                # group order: high blocks reduce first, so their matmuls
                # unblock early and the layer tail runs through the lightest
                # (lowest-degree) block.
                for t in reversed(range(NB)):
                    xT_ps = ps.tile([128, 128], bf16, tag="xt_ps")
                    nc.tensor.transpose(xT_ps[:], x[:, t, :], ident[:])
                    xT = xtp.tile([128, 128], bf16, tag="xt")
                    nc.scalar.activation(xT[:], xT_ps[:], Act.Copy)
                    yv_ps = ps.tile([128, 2 * F], fp32, tag="yv_ps")
                    nc.tensor.matmul(yv_ps[:], lhsT=xT[:], rhs=W[:],
                                     start=True, stop=False)
                    nc.tensor.matmul(yv_ps[:], lhsT=ones[:], rhs=cb2[:],
                                     start=False, stop=True)
                    y_sb = yp.tile([128, F], bf16, tag="y")
                    nc.scalar.activation(y_sb[:], yv_ps[:, 0:F], Act.Copy)
                    if t == 0:
                        nc.vector.memset(y_sb[0:N_PHANTOM, :], NEG)
                    nc.sync.dma_start(y_ag_in[t * 128 : (t + 1) * 128, :], y_sb[:])
                    nc.scalar.activation(v[:, t, :], yv_ps[:, F : 2 * F], Act.Copy)

                if "ag" not in SKIP:
                    nc.gpsimd.collective_compute(
                        "AllGather",
                        Alu.bypass,
                        replica_groups=[list(range(NCORES))],
                        ins=[y_ag_in.opt()],
                        outs=[y_all.opt()],
                    )
                else:
                    nc.sync.dma_start(y_all[0:NPCP, :], y_ag_in[:])

                # ---- gather + segment-max phase ----
                if l < NL - 1:
                    x_next = xp.tile([128, NB, F], bf16, tag="x")
                for (b0, nbl, aoff, acnt, boff, bcnt) in reversed(groups):
                    gA = gap.tile([128, GMAX, F], bf16, tag="ga")
                    gB = gbp.tile([128, GMAX, F], bf16, tag="gb")
                    if "gather" in SKIP:
                        nc.vector.memset(gA[:], 0.0)
                        nc.vector.memset(gB[:], 0.0)
                    else:
                        # 8-chunk calls (65 descs/engine-lane) are known-safe;
                        # 24- and 40-chunk calls hang the SWDGE ring
                        for o in range(0, acnt, 8):
                            n = min(8, acnt - o)
                            nc.gpsimd.dma_gather(
                                gA[:, o : o + n, :], y_all[:, :],
                                idxA[:, (aoff + o) * 8 : (aoff + o + n) * 8],
                                n * 128, n * 128, F,
                                queue_num=qn[0] % 4,
                            )
                            qn[0] += 1
                        for o in range(0, bcnt, 8):
                            n = min(8, bcnt - o)
                            nc.gpsimd.dma_gather(
                                gB[:, o : o + n, :], y_all[BASE_B:, :],
                                idxB[:, (boff + o) * 8 : (boff + o + n) * 8],
                                n * 128, n * 128, F,
                                queue_num=qn[0] % 4,
                            )
                            qn[0] += 1
                    for b in reversed(range(b0, b0 + nbl)):
                        ka = int(cbA[b] - cbA[b0])
                        kb = int(cbB[b] - cbB[b0])
                        ha, hb = int(KA[b]), int(KB[b])
                        # in-place max tree per window over contiguous slices
                        for gt, o, c in ((gA, ka, ha), (gB, kb, hb)):
                            while c > 1:
                                h = c // 2
                                nc.vector.tensor_tensor(
                                    out=gt[:, o : o + h, :],
                                    in0=gt[:, o : o + h, :],
                                    in1=gt[:, o + c - h : o + c, :],
                                    op=Alu.max)
                                c -= h
                        tv = tp.tile([128, F], fp32, tag="tv")
                        if ha > 0 and hb > 0:
                            tM = tp.tile([128, F], bf16, tag="tm")
                            nc.vector.tensor_tensor(out=tM[:], in0=gA[:, ka, :],
                                                    in1=gB[:, kb, :], op=Alu.max)
                            nc.vector.tensor_tensor(out=tv[:], in0=tM[:],
                                                    in1=v[:, b, :], op=Alu.add)
                        elif ha > 0 or hb > 0:
                            srcg = gA[:, ka, :] if ha > 0 else gB[:, kb, :]
                            nc.vector.tensor_tensor(out=tv[:], in0=srcg,
                                                    in1=v[:, b, :], op=Alu.add)
                        else:
                            nc.vector.memset(tv[:], NEG)
                        # relu on DVE: keeps the scalar engine free so the
                        # next layer's matmul-phase copies overlap this phase
                        # (tensor_tensor vs a zeros tile — tensor_scalar with
                        # a float const takes a 5x slower DVE path)
                        if l < NL - 1:
                            nc.vector.tensor_tensor(out=x_next[:, b, :], in0=tv[:],
                                                    in1=zeros[:], op=Alu.max)
                        else:
                            xo = tp.tile([128, F], fp32, tag="xo")
                            nc.vector.tensor_tensor(out=xo[:], in0=tv[:],
                                                    in1=zeros[:], op=Alu.max)
                            nc.sync.dma_start(
                                xout[b * 128 : (b + 1) * 128, :], xo[:])
                if l < NL - 1:
                    x = x_next

    nc.compile()
    return nc


# ----------------------------------------------------------------------------
# numpy emulation of the device dataflow (for validating prep structures)
# ----------------------------------------------------------------------------

def _emulate(g, feats_dev, wcat, cb):
    KA, KB = g["KA"], g["KB"]

    def b16(a):
        return a.astype(BF16).astype(np.float32)

    x = feats_dev.astype(np.float32)  # [NCORES, NPCP, F] sigma-ordered (bf16 values)
    w32 = np.asarray(wcat, np.float32)
    for l in range(NL):
        y_sh = b16(np.einsum("cnf,fk->cnk", x, w32[l, :, :F]))
        v = b16(np.einsum("cnf,fk->cnk", x, w32[l, :, F:]) + cb[l])
        y_sh[:, :N_PHANTOM, :] = NEG
        table = y_sh.reshape(NTAB, F)
        xn = np.empty_like(x)
        for c in range(NCORES):
            gA = table[g["idxA_flat"][c].astype(np.int64)].reshape(g["CA"], 128, F)
            gB = table[BASE_B + g["idxB_flat"][c].astype(np.int64)].reshape(g["CB"], 128, F)
            for b in range(NB):
                a0, b0 = g["cbA"][b], g["cbB"][b]
                parts = []
                if KA[b] > 0:
                    parts.append(gA[a0 : a0 + KA[b]].max(0))
                if KB[b] > 0:
                    parts.append(gB[b0 : b0 + KB[b]].max(0))
                agg = np.full((128, F), NEG, np.float32) if not parts else (
                    parts[0] if len(parts) == 1 else np.maximum(*parts))
                xn[c, b * 128 : (b + 1) * 128] = np.maximum(
                    agg + v[c, b * 128 : (b + 1) * 128], 0.0)
        x = b16(xn) if l < NL - 1 else xn
    return x


def _make_in_maps(g, feats_dev, wcat, cb2):
    in_maps = []
    for c in range(NCORES):
        in_maps.append({
            "xin": np.ascontiguousarray(feats_dev[c]),
            "idxA": np.ascontiguousarray(g["idxA"][c]),
            "idxB": np.ascontiguousarray(g["idxB"][c]),
            "wcat": wcat,
            "cb2": cb2,
        })
    return in_maps


def _feats_dev(g, feats):
    feats = np.asarray(feats, np.float32)
    fd = np.zeros((NCORES, NPCP, F), BF16)
    core = np.arange(N) // NPC
    fd[core, g["pos"]] = feats.astype(BF16)
    return fd


def _assemble(g, results):
    out_sh = np.stack([np.asarray(r["xout"], np.float32) for r in results])
    core = np.arange(N) // NPC
    return np.ascontiguousarray(out_sh[core, g["pos"]])


def run(feats, src, dst, theta_w, theta_b, phi_w, phi_b, trace=False):
    from concourse.bass_utils import run_bass_kernel_spmd

    key = (src.tobytes()[:64], dst.tobytes()[:64], len(src))
    if _cache.get("graph_key") != key:
        _cache.clear()
        _cache["graph"] = _prep_graph(src, dst)
        _cache["graph_key"] = key
    g = _cache["graph"]
    if "nc" not in _cache:
        _cache["nc"] = _build_kernel(g)
    nc = _cache["nc"]

    wcat, cb2, cb = _prep_weights(theta_w, theta_b, phi_w, phi_b)
    feats_dev = _feats_dev(g, feats)
    in_maps = _make_in_maps(g, feats_dev, wcat, cb2)
    res = run_bass_kernel_spmd(nc, in_maps, core_ids=list(range(NCORES)),
                               trace=trace)
    out = _assemble(g, res.results)
    return out, res


def kernel(feats, src, dst, theta_w, theta_b, phi_w, phi_b):
    out, _ = run(feats, src, dst, theta_w, theta_b, phi_w, phi_b)
    return out


# revision 24
# speedup vs baseline: 1.0562x; 1.0562x over previous
"""EdgeConv GNN (4 layers) on 8 Trainium2 NeuronCores.

Algebraic restructure: with y = x @ theta_w.T and
v = x @ (phi_w - theta_w).T + (phi_b + theta_b),
    msg_e = theta(x[src]-x[dst]) + theta_b + phi(x[dst]) + phi_b
          = y[src] + v[dst]
and since v[dst] is constant within a dst segment:
    out = relu(v + segment_max(y[src], dst))
(nodes with no in-edges come out of segment_max at -1e30 -> relu -> 0,
matching the reference's where(isneginf, 0) + relu).

Distribution: nodes sharded by dst across 8 cores (graph parallel).
Each layer: per-core bf16 matmuls produce its y-shard -> AllGather the
bf16 y table -> SWDGE dma_gather of y rows by src in dst-sorted slot
order -> per-block max-tree reduce on DVE (contiguous tensor_tensor,
not strided tensor_reduce) -> + v -> relu.

dma_gather indices are int16 (<= 32767) so the 50176-row table is
addressed through two windows: A = rows [0, 31360) (src cores 0-4) and
B = rows [18816, 50176) (src cores 3-7). Src cores 3-4 edges are flex
(either window); a per-block joint optimization assigns flex edges to
minimize sum_b(KA[b] + KB[b]) where K* are per-128-node-block degree
caps (shared across the 8 cores: single SPMD instruction stream).
"""

import numpy as np
import ml_dtypes

BF16 = ml_dtypes.bfloat16

N = 50000
NCORES = 8
NPC = 6250            # real nodes per core
NPCP = 6272           # padded nodes per core (49 * 128)
F = 128
NL = 4
NB = NPCP // 128      # 49 blocks per core
NTAB = NCORES * NPCP  # 50176 table rows
BASE_B = 3 * NPCP     # 18816: window B base row (cores 3-7)
N_PHANTOM = NPCP - NPC
GMAX = 40             # max chunks per gather call (per window)
NEG = -1.0e30
SKIP = set()          # debug: subset of {"gather", "reduce", "ag", "mm"}

_cache = {}


# ----------------------------------------------------------------------------
# host-side graph preprocessing
# ----------------------------------------------------------------------------

def _prep_graph(src, dst):
    src = np.asarray(src).astype(np.int64)
    dst = np.asarray(dst).astype(np.int64)
    s_core = src // NPC
    d_core = dst // NPC
    core = np.arange(N) // NPC

    fixedA = s_core <= 2          # rows < 18816: window A only
    fixedB = s_core >= 5          # rows >= 31360: window B only
    flexm = ~fixedA & ~fixedB     # src cores 3-4: rows in both windows
    fA = np.bincount(dst[fixedA], minlength=N)
    fB = np.bincount(dst[fixedB], minlength=N)
    fx = np.bincount(dst[flexm], minlength=N)

    # iterate: (node order by balanced degrees) <-> (per-block joint split)
    kA = np.clip((fB - fA + fx + 1) // 2, 0, fx)
    best = None
    for _ in range(3):
        dA, dB = fA + kA, fB + fx - kA
        pos = np.empty(N, np.int64)
        for c in range(NCORES):
            ids = np.arange(c * NPC, (c + 1) * NPC)
            order = np.lexsort((dA[ids] + dB[ids], np.maximum(dA[ids], dB[ids])))
            pos[ids[order]] = N_PHANTOM + np.arange(NPC)
        blk = pos // 128
        KA = np.zeros(NB, np.int64)
        KB = np.zeros(NB, np.int64)
        kA_new = np.zeros(N, np.int64)
        for b in range(NB):
            m = blk == b
            a, x, bb = fA[m], fx[m], fB[m]
            lo, hi = int(a.max()), int((a + x).max())
            bt = None
            for tA in range(lo, hi + 1):
                kab = np.minimum(x, tA - a)
                tB = int((bb + x - kab).max())
                if bt is None or tA + tB < bt[0] + bt[1]:
                    bt = (tA, tB)
            tA, tB = bt
            kmin = np.maximum(0, x - (tB - bb))
            kmax = np.minimum(x, tA - a)
            kA_new[m] = (kmin + kmax) // 2
            KA[b], KB[b] = tA, tB
        tot = int(KA.sum() + KB.sum())
        if best is None or tot < best[0]:
            best = (tot, KA.copy(), KB.copy(), pos.copy(), kA_new.copy())
        kA = kA_new
    _, KA, KB, pos, kA = best

    sig = core * NPCP + pos       # orig node -> table row
    blk = pos // 128
    lane = pos % 128
    cbA = np.r_[0, np.cumsum(KA)]
    cbB = np.r_[0, np.cumsum(KB)]
    CA, CB = int(cbA[-1]), int(cbB[-1])
    assert KA.max() <= GMAX and KB.max() <= GMAX, (KA.max(), KB.max())

    # edge side: fixed by src core; flex edges ranked within their dst group
    sideA = fixedA.copy()
    fe = np.flatnonzero(flexm)
    fe = fe[np.argsort(dst[fe], kind="stable")]
    dsf = dst[fe]
    starts = np.r_[0, np.flatnonzero(np.diff(dsf)) + 1]
    runlen = np.diff(np.r_[starts, len(dsf)])
    rank = np.arange(len(dsf)) - np.repeat(starts, runlen)
    sideA[fe[rank < kA[dsf]]] = True

    # slot arrays (per core); dummy slots use idx 0 (phantom/NEG row in
    # both windows: row 0 and row BASE_B = core 3 pos 0 are phantoms)
    idxA = np.zeros((NCORES, CA * 128), np.int16)
    idxB = np.zeros((NCORES, CB * 128), np.int16)
    for side, idx_arr, cb, base, cap in (
        (True, idxA, cbA, 0, KA), (False, idxB, cbB, BASE_B, KB)
    ):
        e = np.flatnonzero(sideA == side)
        e = e[np.argsort(dst[e], kind="stable")]
        de = dst[e]
        starts = np.r_[0, np.flatnonzero(np.diff(de)) + 1]
        runlen = np.diff(np.r_[starts, len(de)])
        rank = np.arange(len(de)) - np.repeat(starts, runlen)
        assert (rank < cap[blk[de]]).all()
        slot = (cb[blk[de]] + rank) * 128 + lane[de]
        val = sig[src[e]] - base
        assert val.min() >= 0 and val.max() < 32768, (val.min(), val.max())
        idx_arr[d_core[e], slot] = val.astype(np.int16)

    # wrap indices: [n] -> [128, n//16] int16, replicated across 8 groups of 16
    def wrap(a):
        n = a.shape[1]
        w = a.reshape(NCORES, n // 16, 16).transpose(0, 2, 1)  # [c, 16, n/16]
        return np.ascontiguousarray(
            np.broadcast_to(w[:, None, :, :], (NCORES, 8, 16, n // 16))
        ).reshape(NCORES, 128, n // 16)

    # gather groups: consecutive blocks, chunk budget GMAX per window
    groups = []
    b0 = 0
    while b0 < NB:
        nb = 1
        while (
            b0 + nb < NB
            and cbA[b0 + nb + 1] - cbA[b0] <= GMAX
            and cbB[b0 + nb + 1] - cbB[b0] <= GMAX
        ):
            nb += 1
        groups.append((b0, nb, int(cbA[b0]), int(cbA[b0 + nb] - cbA[b0]),
                       int(cbB[b0]), int(cbB[b0 + nb] - cbB[b0])))
        b0 += nb

    return dict(
        sig=sig, pos=pos, KA=KA, KB=KB, cbA=cbA, cbB=cbB, CA=CA, CB=CB,
        idxA=wrap(idxA), idxB=wrap(idxB), groups=groups,
        idxA_flat=idxA, idxB_flat=idxB,
    )


def _prep_weights(theta_w, theta_b, phi_w, phi_b):
    theta_w = np.asarray(theta_w, np.float32)
    phi_w = np.asarray(phi_w, np.float32)
    cb = (np.asarray(theta_b, np.float32) + np.asarray(phi_b, np.float32))
    wcat = np.concatenate(
        [theta_w.transpose(0, 2, 1), (phi_w - theta_w).transpose(0, 2, 1)], axis=2
    )  # [NL, 128(in), 256(out: y|v)]
    # rank-1 bias row: [NL, 1, 256] = [zeros(128) | cb] — accumulated into the
    # yv PSUM tile via a K=1 matmul with a ones column (frees DVE in mm phase)
    cb2 = np.concatenate([np.zeros((NL, 1, F), np.float32), cb[:, None, :]], axis=2)
    return (np.ascontiguousarray(wcat.astype(BF16)),
            np.ascontiguousarray(cb2.astype(BF16)),
            np.ascontiguousarray(cb))


# ----------------------------------------------------------------------------
# device kernel
# ----------------------------------------------------------------------------

def _build_kernel(g):
    import concourse.bacc as bacc
    import concourse.mybir as mybir
    import concourse.tile as tile
    from concourse.masks import make_identity

    KA, KB, groups = g["KA"], g["KB"], g["groups"]
    cbA, cbB = g["cbA"], g["cbB"]
    CA, CB = g["CA"], g["CB"]

    nc = bacc.Bacc("TRN2", target_bir_lowering=False, debug=False,
                   num_devices=NCORES, num_swdge_queues=4)

    bf16 = mybir.dt.bfloat16
    fp32 = mybir.dt.float32
    Alu = mybir.AluOpType
    Act = mybir.ActivationFunctionType

    xin = nc.dram_tensor("xin", [NPCP, F], bf16, kind="ExternalInput")
    idxA_in = nc.dram_tensor("idxA", [128, CA * 8], mybir.dt.int16, kind="ExternalInput")
    idxB_in = nc.dram_tensor("idxB", [128, CB * 8], mybir.dt.int16, kind="ExternalInput")
    wcat_in = nc.dram_tensor("wcat", [NL, F, 2 * F], bf16, kind="ExternalInput")
    cb2_in = nc.dram_tensor("cb2", [NL, 1, 2 * F], bf16, kind="ExternalInput")
    xout = nc.dram_tensor("xout", [NPCP, F], fp32, kind="ExternalOutput")

    with tile.TileContext(nc) as tc:
        with (
            tc.tile_pool(name="const", bufs=1) as constp,
            tc.tile_pool(name="xp", bufs=2) as xp,
            tc.tile_pool(name="vp", bufs=2) as vp,
            tc.tile_pool(name="wp", bufs=2) as wp,
            tc.tile_pool(name="yp", bufs=3) as yp,
            tc.tile_pool(name="xtp", bufs=3) as xtp,
            tc.tile_pool(name="ga", bufs=4) as gap,
            tc.tile_pool(name="gb", bufs=4) as gbp,
            tc.tile_pool(name="tp", bufs=4) as tp,
            tc.tile_pool(name="ps", bufs=4, space="PSUM") as ps,
            tc.tile_pool(name="dram", bufs=2, space="DRAM") as dram,
        ):
            ident = constp.tile([128, 128], bf16)
            make_identity(nc, ident[:])
            ones = constp.tile([1, 128], bf16)
            nc.vector.memset(ones[:], 1.0)
            zeros = constp.tile([128, F], fp32)
            nc.vector.memset(zeros[:], 0.0)
            qn = [0]  # SWDGE queue round-robin across gather calls
            idxA = constp.tile([128, CA * 8], mybir.dt.int16)
            idxB = constp.tile([128, CB * 8], mybir.dt.int16)
            nc.sync.dma_start(idxA[:], idxA_in[:])
            nc.sync.dma_start(idxB[:], idxB_in[:])

            x = xp.tile([128, NB, F], bf16, tag="x")
            nc.sync.dma_start(x[:], xin.rearrange("(b p) f -> p b f", p=128))

            for l in range(NL):
                W = wp.tile([128, 2 * F], bf16, tag="w")
                nc.sync.dma_start(W[:], wcat_in[l])
                cb2 = wp.tile([1, 2 * F], bf16, tag="cb2")
                nc.sync.dma_start(cb2[:], cb2_in[l])

                y_ag_in = dram.tile([NPCP, F], bf16, tag="yag")
                y_all = dram.tile([NTAB, F], bf16, tag="yall", addr_space="Shared")

                v = vp.tile([128, NB, F], bf16, tag="v")

                # ---- matmul phase: y (table, bf16) and v; DVE-free so the
                # vector engine stays on the (previous layer's) reduce work
                for t in range(NB):
                    xT_ps = ps.tile([128, 128], bf16, tag="xt_ps")
                    nc.tensor.transpose(xT_ps[:], x[:, t, :], ident[:])
                    xT = xtp.tile([128, 128], bf16, tag="xt")
                    nc.scalar.activation(xT[:], xT_ps[:], Act.Copy)
                    yv_ps = ps.tile([128, 2 * F], fp32, tag="yv_ps")
                    nc.tensor.matmul(yv_ps[:], lhsT=xT[:], rhs=W[:],
                                     start=True, stop=False)
                    nc.tensor.matmul(yv_ps[:], lhsT=ones[:], rhs=cb2[:],
                                     start=False, stop=True)
                    y_sb = yp.tile([128, F], bf16, tag="y")
                    nc.scalar.activation(y_sb[:], yv_ps[:, 0:F], Act.Copy)
                    if t == 0:
                        nc.vector.memset(y_sb[0:N_PHANTOM, :], NEG)
                    nc.sync.dma_start(y_ag_in[t * 128 : (t + 1) * 128, :], y_sb[:])
                    nc.scalar.activation(v[:, t, :], yv_ps[:, F : 2 * F], Act.Copy)

                if "ag" not in SKIP:
                    nc.gpsimd.collective_compute(
                        "AllGather",
                        Alu.bypass,
                        replica_groups=[list(range(NCORES))],
                        ins=[y_ag_in.opt()],
                        outs=[y_all.opt()],
                    )
                else:
                    nc.sync.dma_start(y_all[0:NPCP, :], y_ag_in[:])

                # ---- gather + segment-max phase ----
                if l < NL - 1:
                    x_next = xp.tile([128, NB, F], bf16, tag="x")
                for (b0, nbl, aoff, acnt, boff, bcnt) in groups:
                    gA = gap.tile([128, GMAX, F], bf16, tag="ga")
                    gB = gbp.tile([128, GMAX, F], bf16, tag="gb")
                    if "gather" in SKIP:
                        nc.vector.memset(gA[:], 0.0)
                        nc.vector.memset(gB[:], 0.0)
                    else:
                        # SWDGE ring fits only ~8 chunks per call: 65
                        # descs/lane works, 97+ (12-chunk) hangs the device
                        for o in range(0, acnt, 8):
                            n = min(8, acnt - o)
                            nc.gpsimd.dma_gather(
                                gA[:, o : o + n, :], y_all[:, :],
                                idxA[:, (aoff + o) * 8 : (aoff + o + n) * 8],
                                n * 128, n * 128, F,
                                queue_num=qn[0] % 4,
                            )
                            qn[0] += 1
                        for o in range(0, bcnt, 8):
                            n = min(8, bcnt - o)
                            nc.gpsimd.dma_gather(
                                gB[:, o : o + n, :], y_all[BASE_B:, :],
                                idxB[:, (boff + o) * 8 : (boff + o + n) * 8],
                                n * 128, n * 128, F,
                                queue_num=qn[0] % 4,
                            )
                            qn[0] += 1
                    for b in range(b0, b0 + nbl):
                        ka = int(cbA[b] - cbA[b0])
                        kb = int(cbB[b] - cbB[b0])
                        ha, hb = int(KA[b]), int(KB[b])
                        # in-place max tree per window over contiguous slices
                        for gt, o, c in ((gA, ka, ha), (gB, kb, hb)):
                            while c > 1:
                                h = c // 2
                                nc.vector.tensor_tensor(
                                    out=gt[:, o : o + h, :],
                                    in0=gt[:, o : o + h, :],
                                    in1=gt[:, o + c - h : o + c, :],
                                    op=Alu.max)
                                c -= h
                        tv = tp.tile([128, F], fp32, tag="tv")
                        if ha > 0 and hb > 0:
                            tM = tp.tile([128, F], bf16, tag="tm")
                            nc.vector.tensor_tensor(out=tM[:], in0=gA[:, ka, :],
                                                    in1=gB[:, kb, :], op=Alu.max)
                            nc.vector.tensor_tensor(out=tv[:], in0=tM[:],
                                                    in1=v[:, b, :], op=Alu.add)
                        elif ha > 0 or hb > 0:
                            srcg = gA[:, ka, :] if ha > 0 else gB[:, kb, :]
                            nc.vector.tensor_tensor(out=tv[:], in0=srcg,
                                                    in1=v[:, b, :], op=Alu.add)
                        else:
                            nc.vector.memset(tv[:], NEG)
                        # relu on DVE: keeps the scalar engine free so the
                        # next layer's matmul-phase copies overlap this phase
                        # (tensor_tensor vs a zeros tile — tensor_scalar with
                        # a float const takes a 5x slower DVE path)
                        if l < NL - 1:
                            nc.vector.tensor_tensor(out=x_next[:, b, :], in0=tv[:],
                                                    in1=zeros[:], op=Alu.max)
                        else:
                            xo = tp.tile([128, F], fp32, tag="xo")
                            nc.vector.tensor_tensor(out=xo[:], in0=tv[:],
                                                    in1=zeros[:], op=Alu.max)
                            nc.sync.dma_start(
                                xout[b * 128 : (b + 1) * 128, :], xo[:])
                if l < NL - 1:
                    x = x_next

    nc.compile()
    return nc


# ----------------------------------------------------------------------------
# numpy emulation of the device dataflow (for validating prep structures)
# ----------------------------------------------------------------------------

def _emulate(g, feats_dev, wcat, cb):
    KA, KB = g["KA"], g["KB"]

    def b16(a):
        return a.astype(BF16).astype(np.float32)

    x = feats_dev.astype(np.float32)  # [NCORES, NPCP, F] sigma-ordered (bf16 values)
    w32 = np.asarray(wcat, np.float32)
    for l in range(NL):
        y_sh = b16(np.einsum("cnf,fk->cnk", x, w32[l, :, :F]))
        v = b16(np.einsum("cnf,fk->cnk", x, w32[l, :, F:]) + cb[l])
        y_sh[:, :N_PHANTOM, :] = NEG
        table = y_sh.reshape(NTAB, F)
        xn = np.empty_like(x)
        for c in range(NCORES):
            gA = table[g["idxA_flat"][c].astype(np.int64)].reshape(g["CA"], 128, F)
            gB = table[BASE_B + g["idxB_flat"][c].astype(np.int64)].reshape(g["CB"], 128, F)
            for b in range(NB):
                a0, b0 = g["cbA"][b], g["cbB"][b]
                parts = []
                if KA[b] > 0:
                    parts.append(gA[a0 : a0 + KA[b]].max(0))
                if KB[b] > 0:
                    parts.append(gB[b0 : b0 + KB[b]].max(0))
                agg = np.full((128, F), NEG, np.float32) if not parts else (
                    parts[0] if len(parts) == 1 else np.maximum(*parts))
                xn[c, b * 128 : (b + 1) * 128] = np.maximum(
                    agg + v[c, b * 128 : (b + 1) * 128], 0.0)
        x = b16(xn) if l < NL - 1 else xn
    return x


def _make_in_maps(g, feats_dev, wcat, cb2):
    in_maps = []
    for c in range(NCORES):
        in_maps.append({
            "xin": np.ascontiguousarray(feats_dev[c]),
            "idxA": np.ascontiguousarray(g["idxA"][c]),
            "idxB": np.ascontiguousarray(g["idxB"][c]),
            "wcat": wcat,
            "cb2": cb2,
        })
    return in_maps


def _feats_dev(g, feats):
    feats = np.asarray(feats, np.float32)
    fd = np.zeros((NCORES, NPCP, F), BF16)
    core = np.arange(N) // NPC
    fd[core, g["pos"]] = feats.astype(BF16)
    return fd


def _assemble(g, results):
    out_sh = np.stack([np.asarray(r["xout"], np.float32) for r in results])
    core = np.arange(N) // NPC
    return np.ascontiguousarray(out_sh[core, g["pos"]])


def run(feats, src, dst, theta_w, theta_b, phi_w, phi_b, trace=False):
    from concourse.bass_utils import run_bass_kernel_spmd

    key = (src.tobytes()[:64], dst.tobytes()[:64], len(src))
    if _cache.get("graph_key") != key:
        _cache.clear()
        _cache["graph"] = _prep_graph(src, dst)
        _cache["graph_key"] = key
    g = _cache["graph"]
    if "nc" not in _cache:
        _cache["nc"] = _build_kernel(g)
    nc = _cache["nc"]

    wcat, cb2, cb = _prep_weights(theta_w, theta_b, phi_w, phi_b)
    feats_dev = _feats_dev(g, feats)
    in_maps = _make_in_maps(g, feats_dev, wcat, cb2)
    res = run_bass_kernel_spmd(nc, in_maps, core_ids=list(range(NCORES)),
                               trace=trace)
    out = _assemble(g, res.results)
    return out, res


def kernel(feats, src, dst, theta_w, theta_b, phi_w, phi_b):
    out, _ = run(feats, src, dst, theta_w, theta_b, phi_w, phi_b)
    return out


# revision 25
# speedup vs baseline: 1.0723x; 1.0152x over previous
"""EdgeConv GNN (4 layers) on 8 Trainium2 NeuronCores.

Algebraic restructure: with y = x @ theta_w.T and
v = x @ (phi_w - theta_w).T + (phi_b + theta_b),
    msg_e = theta(x[src]-x[dst]) + theta_b + phi(x[dst]) + phi_b
          = y[src] + v[dst]
and since v[dst] is constant within a dst segment:
    out = relu(v + segment_max(y[src], dst))
(nodes with no in-edges come out of segment_max at -1e30 -> relu -> 0,
matching the reference's where(isneginf, 0) + relu).

Distribution: nodes sharded by dst across 8 cores (graph parallel).
Each layer: per-core bf16 matmuls produce its y-shard -> AllGather the
bf16 y table -> SWDGE dma_gather of y rows by src in dst-sorted slot
order -> per-block max-tree reduce on DVE (contiguous tensor_tensor,
not strided tensor_reduce) -> + v -> relu.

dma_gather indices are int16 (<= 32767) so the 50176-row table is
addressed through two windows: A = rows [0, 31360) (src cores 0-4) and
B = rows [18816, 50176) (src cores 3-7). Src cores 3-4 edges are flex
(either window); a per-block joint optimization assigns flex edges to
minimize sum_b(KA[b] + KB[b]) where K* are per-128-node-block degree
caps (shared across the 8 cores: single SPMD instruction stream).
"""

import numpy as np
import ml_dtypes

BF16 = ml_dtypes.bfloat16

N = 50000
NCORES = 8
NPC = 6250            # real nodes per core
NPCP = 6272           # padded nodes per core (49 * 128)
F = 128
NL = 4
NB = NPCP // 128      # 49 blocks per core
NTAB = NCORES * NPCP  # 50176 table rows
BASE_B = 3 * NPCP     # 18816: window B base row (cores 3-7)
N_PHANTOM = NPCP - NPC
GMAX = 40             # max chunks per gather call (per window)
NEG = -1.0e30
SKIP = set()          # debug: subset of {"gather", "reduce", "ag", "mm"}

_cache = {}


# ----------------------------------------------------------------------------
# host-side graph preprocessing
# ----------------------------------------------------------------------------

def _prep_graph(src, dst):
    src = np.asarray(src).astype(np.int64)
    dst = np.asarray(dst).astype(np.int64)
    s_core = src // NPC
    d_core = dst // NPC
    core = np.arange(N) // NPC

    fixedA = s_core <= 2          # rows < 18816: window A only
    fixedB = s_core >= 5          # rows >= 31360: window B only
    flexm = ~fixedA & ~fixedB     # src cores 3-4: rows in both windows
    fA = np.bincount(dst[fixedA], minlength=N)
    fB = np.bincount(dst[fixedB], minlength=N)
    fx = np.bincount(dst[flexm], minlength=N)

    # iterate: (node order by balanced degrees) <-> (per-block joint split)
    kA = np.clip((fB - fA + fx + 1) // 2, 0, fx)
    best = None
    for _ in range(3):
        dA, dB = fA + kA, fB + fx - kA
        pos = np.empty(N, np.int64)
        for c in range(NCORES):
            ids = np.arange(c * NPC, (c + 1) * NPC)
            order = np.lexsort((dA[ids] + dB[ids], np.maximum(dA[ids], dB[ids])))
            pos[ids[order]] = N_PHANTOM + np.arange(NPC)
        blk = pos // 128
        KA = np.zeros(NB, np.int64)
        KB = np.zeros(NB, np.int64)
        kA_new = np.zeros(N, np.int64)
        for b in range(NB):
            m = blk == b
            a, x, bb = fA[m], fx[m], fB[m]
            lo, hi = int(a.max()), int((a + x).max())
            bt = None
            for tA in range(lo, hi + 1):
                kab = np.minimum(x, tA - a)
                tB = int((bb + x - kab).max())
                if bt is None or tA + tB < bt[0] + bt[1]:
                    bt = (tA, tB)
            tA, tB = bt
            kmin = np.maximum(0, x - (tB - bb))
            kmax = np.minimum(x, tA - a)
            kA_new[m] = (kmin + kmax) // 2
            KA[b], KB[b] = tA, tB
        tot = int(KA.sum() + KB.sum())
        if best is None or tot < best[0]:
            best = (tot, KA.copy(), KB.copy(), pos.copy(), kA_new.copy())
        kA = kA_new
    _, KA, KB, pos, kA = best

    sig = core * NPCP + pos       # orig node -> table row
    blk = pos // 128
    lane = pos % 128
    cbA = np.r_[0, np.cumsum(KA)]
    cbB = np.r_[0, np.cumsum(KB)]
    CA, CB = int(cbA[-1]), int(cbB[-1])
    assert KA.max() <= GMAX and KB.max() <= GMAX, (KA.max(), KB.max())

    # edge side: fixed by src core; flex edges ranked within their dst group
    sideA = fixedA.copy()
    fe = np.flatnonzero(flexm)
    fe = fe[np.argsort(dst[fe], kind="stable")]
    dsf = dst[fe]
    starts = np.r_[0, np.flatnonzero(np.diff(dsf)) + 1]
    runlen = np.diff(np.r_[starts, len(dsf)])
    rank = np.arange(len(dsf)) - np.repeat(starts, runlen)
    sideA[fe[rank < kA[dsf]]] = True

    # slot arrays (per core); dummy slots use idx 0 (phantom/NEG row in
    # both windows: row 0 and row BASE_B = core 3 pos 0 are phantoms)
    idxA = np.zeros((NCORES, CA * 128), np.int16)
    idxB = np.zeros((NCORES, CB * 128), np.int16)
    for side, idx_arr, cb, base, cap in (
        (True, idxA, cbA, 0, KA), (False, idxB, cbB, BASE_B, KB)
    ):
        e = np.flatnonzero(sideA == side)
        e = e[np.argsort(dst[e], kind="stable")]
        de = dst[e]
        starts = np.r_[0, np.flatnonzero(np.diff(de)) + 1]
        runlen = np.diff(np.r_[starts, len(de)])
        rank = np.arange(len(de)) - np.repeat(starts, runlen)
        assert (rank < cap[blk[de]]).all()
        slot = (cb[blk[de]] + rank) * 128 + lane[de]
        val = sig[src[e]] - base
        assert val.min() >= 0 and val.max() < 32768, (val.min(), val.max())
        idx_arr[d_core[e], slot] = val.astype(np.int16)

    # wrap indices: [n] -> [128, n//16] int16, replicated across 8 groups of 16
    def wrap(a):
        n = a.shape[1]
        w = a.reshape(NCORES, n // 16, 16).transpose(0, 2, 1)  # [c, 16, n/16]
        return np.ascontiguousarray(
            np.broadcast_to(w[:, None, :, :], (NCORES, 8, 16, n // 16))
        ).reshape(NCORES, 128, n // 16)

    # gather groups: consecutive blocks, chunk budget GMAX per window
    groups = []
    b0 = 0
    while b0 < NB:
        nb = 1
        while (
            b0 + nb < NB
            and cbA[b0 + nb + 1] - cbA[b0] <= GMAX
            and cbB[b0 + nb + 1] - cbB[b0] <= GMAX
        ):
            nb += 1
        groups.append((b0, nb, int(cbA[b0]), int(cbA[b0 + nb] - cbA[b0]),
                       int(cbB[b0]), int(cbB[b0 + nb] - cbB[b0])))
        b0 += nb

    return dict(
        sig=sig, pos=pos, KA=KA, KB=KB, cbA=cbA, cbB=cbB, CA=CA, CB=CB,
        idxA=wrap(idxA), idxB=wrap(idxB), groups=groups,
        idxA_flat=idxA, idxB_flat=idxB,
    )


def _prep_weights(theta_w, theta_b, phi_w, phi_b):
    theta_w = np.asarray(theta_w, np.float32)
    phi_w = np.asarray(phi_w, np.float32)
    cb = (np.asarray(theta_b, np.float32) + np.asarray(phi_b, np.float32))
    wcat = np.concatenate(
        [theta_w.transpose(0, 2, 1), (phi_w - theta_w).transpose(0, 2, 1)], axis=2
    )  # [NL, 128(in), 256(out: y|v)]
    # rank-1 bias row: [NL, 1, 256] = [zeros(128) | cb] — accumulated into the
    # yv PSUM tile via a K=1 matmul with a ones column (frees DVE in mm phase)
    cb2 = np.concatenate([np.zeros((NL, 1, F), np.float32), cb[:, None, :]], axis=2)
    return (np.ascontiguousarray(wcat.astype(BF16)),
            np.ascontiguousarray(cb2.astype(BF16)),
            np.ascontiguousarray(cb))


# ----------------------------------------------------------------------------
# device kernel
# ----------------------------------------------------------------------------

def _build_kernel(g):
    import concourse.bacc as bacc
    import concourse.mybir as mybir
    import concourse.tile as tile
    from concourse.masks import make_identity

    KA, KB, groups = g["KA"], g["KB"], g["groups"]
    cbA, cbB = g["cbA"], g["cbB"]
    CA, CB = g["CA"], g["CB"]

    nc = bacc.Bacc("TRN2", target_bir_lowering=False, debug=False,
                   num_devices=NCORES, num_swdge_queues=4)

    bf16 = mybir.dt.bfloat16
    fp32 = mybir.dt.float32
    Alu = mybir.AluOpType
    Act = mybir.ActivationFunctionType

    xin = nc.dram_tensor("xin", [NPCP, F], bf16, kind="ExternalInput")
    idxA_in = nc.dram_tensor("idxA", [128, CA * 8], mybir.dt.int16, kind="ExternalInput")
    idxB_in = nc.dram_tensor("idxB", [128, CB * 8], mybir.dt.int16, kind="ExternalInput")
    wcat_in = nc.dram_tensor("wcat", [NL, F, 2 * F], bf16, kind="ExternalInput")
    cb2_in = nc.dram_tensor("cb2", [NL, 1, 2 * F], bf16, kind="ExternalInput")
    xout = nc.dram_tensor("xout", [NPCP, F], fp32, kind="ExternalOutput")

    with tile.TileContext(nc) as tc:
        with (
            tc.tile_pool(name="const", bufs=1) as constp,
            tc.tile_pool(name="xp", bufs=2) as xp,
            tc.tile_pool(name="vp", bufs=2) as vp,
            tc.tile_pool(name="wp", bufs=2) as wp,
            tc.tile_pool(name="yp", bufs=3) as yp,
            tc.tile_pool(name="xtp", bufs=3) as xtp,
            tc.tile_pool(name="ga", bufs=3) as gap,
            tc.tile_pool(name="gb", bufs=3) as gbp,
            tc.tile_pool(name="tp", bufs=4) as tp,
            tc.tile_pool(name="ps", bufs=4, space="PSUM") as ps,
            tc.tile_pool(name="dram", bufs=2, space="DRAM") as dram,
        ):
            ident = constp.tile([128, 128], bf16)
            make_identity(nc, ident[:])
            ones = constp.tile([1, 128], bf16)
            nc.vector.memset(ones[:], 1.0)
            zeros = constp.tile([128, F], fp32)
            nc.vector.memset(zeros[:], 0.0)
            qn = [0]  # SWDGE queue round-robin across gather calls
            idxA = constp.tile([128, CA * 8], mybir.dt.int16)
            idxB = constp.tile([128, CB * 8], mybir.dt.int16)
            nc.sync.dma_start(idxA[:], idxA_in[:])
            nc.sync.dma_start(idxB[:], idxB_in[:])

            x = xp.tile([128, NB, F], bf16, tag="x")
            nc.sync.dma_start(x[:], xin.rearrange("(b p) f -> p b f", p=128))

            for l in range(NL):
                W = wp.tile([128, 2 * F], bf16, tag="w")
                nc.sync.dma_start(W[:], wcat_in[l])
                cb2 = wp.tile([1, 2 * F], bf16, tag="cb2")
                nc.sync.dma_start(cb2[:], cb2_in[l])

                y_ag_in = dram.tile([NPCP, F], bf16, tag="yag")
                y_all = dram.tile([NTAB, F], bf16, tag="yall", addr_space="Shared")

                v = vp.tile([128, NB, F], bf16, tag="v")

                # ---- matmul phase: y (table, bf16) and v; DVE-free so the
                # vector engine stays on the (previous layer's) reduce work
                for t in range(NB):
                    xT_ps = ps.tile([128, 128], bf16, tag="xt_ps")
                    nc.tensor.transpose(xT_ps[:], x[:, t, :], ident[:])
                    xT = xtp.tile([128, 128], bf16, tag="xt")
                    nc.scalar.activation(xT[:], xT_ps[:], Act.Copy)
                    yv_ps = ps.tile([128, 2 * F], fp32, tag="yv_ps")
                    nc.tensor.matmul(yv_ps[:], lhsT=xT[:], rhs=W[:],
                                     start=True, stop=False)
                    nc.tensor.matmul(yv_ps[:], lhsT=ones[:], rhs=cb2[:],
                                     start=False, stop=True)
                    y_sb = yp.tile([128, F], bf16, tag="y")
                    nc.scalar.activation(y_sb[:], yv_ps[:, 0:F], Act.Copy)
                    if t == 0:
                        nc.vector.memset(y_sb[0:N_PHANTOM, :], NEG)
                    nc.sync.dma_start(y_ag_in[t * 128 : (t + 1) * 128, :], y_sb[:])
                    nc.scalar.activation(v[:, t, :], yv_ps[:, F : 2 * F], Act.Copy)

                if "ag" not in SKIP:
                    nc.gpsimd.collective_compute(
                        "AllGather",
                        Alu.bypass,
                        replica_groups=[list(range(NCORES))],
                        ins=[y_ag_in.opt()],
                        outs=[y_all.opt()],
                    )
                else:
                    nc.sync.dma_start(y_all[0:NPCP, :], y_ag_in[:])

                # ---- gather + segment-max phase ----
                if l < NL - 1:
                    x_next = xp.tile([128, NB, F], bf16, tag="x")
                for (b0, nbl, aoff, acnt, boff, bcnt) in groups:
                    gA = gap.tile([128, GMAX, F], bf16, tag="ga")
                    gB = gbp.tile([128, GMAX, F], bf16, tag="gb")
                    if "gather" in SKIP:
                        nc.vector.memset(gA[:], 0.0)
                        nc.vector.memset(gB[:], 0.0)
                    else:
                        # SWDGE ring fits only ~8 chunks per call: 65
                        # descs/lane works, 97+ (12-chunk) hangs the device
                        for o in range(0, acnt, 8):
                            n = min(8, acnt - o)
                            nc.gpsimd.dma_gather(
                                gA[:, o : o + n, :], y_all[:, :],
                                idxA[:, (aoff + o) * 8 : (aoff + o + n) * 8],
                                n * 128, n * 128, F,
                                queue_num=qn[0] % 4,
                            )
                            qn[0] += 1
                        for o in range(0, bcnt, 8):
                            n = min(8, bcnt - o)
                            nc.gpsimd.dma_gather(
                                gB[:, o : o + n, :], y_all[BASE_B:, :],
                                idxB[:, (boff + o) * 8 : (boff + o + n) * 8],
                                n * 128, n * 128, F,
                                queue_num=qn[0] % 4,
                            )
                            qn[0] += 1
                    for b in range(b0, b0 + nbl):
                        ka = int(cbA[b] - cbA[b0])
                        kb = int(cbB[b] - cbB[b0])
                        ha, hb = int(KA[b]), int(KB[b])
                        # in-place max tree per window over contiguous slices
                        for gt, o, c in ((gA, ka, ha), (gB, kb, hb)):
                            while c > 1:
                                h = c // 2
                                nc.vector.tensor_tensor(
                                    out=gt[:, o : o + h, :],
                                    in0=gt[:, o : o + h, :],
                                    in1=gt[:, o + c - h : o + c, :],
                                    op=Alu.max)
                                c -= h
                        tv = tp.tile([128, F], fp32, tag="tv")
                        if ha > 0 and hb > 0:
                            tM = tp.tile([128, F], bf16, tag="tm")
                            nc.vector.tensor_tensor(out=tM[:], in0=gA[:, ka, :],
                                                    in1=gB[:, kb, :], op=Alu.max)
                            nc.vector.tensor_tensor(out=tv[:], in0=tM[:],
                                                    in1=v[:, b, :], op=Alu.add)
                        elif ha > 0 or hb > 0:
                            srcg = gA[:, ka, :] if ha > 0 else gB[:, kb, :]
                            nc.vector.tensor_tensor(out=tv[:], in0=srcg,
                                                    in1=v[:, b, :], op=Alu.add)
                        else:
                            nc.vector.memset(tv[:], NEG)
                        # relu on DVE: keeps the scalar engine free so the
                        # next layer's matmul-phase copies overlap this phase
                        # (tensor_tensor vs a zeros tile — tensor_scalar with
                        # a float const takes a 5x slower DVE path)
                        if l < NL - 1:
                            nc.vector.tensor_tensor(out=x_next[:, b, :], in0=tv[:],
                                                    in1=zeros[:], op=Alu.max)
                        else:
                            xo = tp.tile([128, F], fp32, tag="xo")
                            nc.vector.tensor_tensor(out=xo[:], in0=tv[:],
                                                    in1=zeros[:], op=Alu.max)
                            nc.sync.dma_start(
                                xout[b * 128 : (b + 1) * 128, :], xo[:])
                if l < NL - 1:
                    x = x_next

    nc.compile()
    return nc


# ----------------------------------------------------------------------------
# numpy emulation of the device dataflow (for validating prep structures)
# ----------------------------------------------------------------------------

def _emulate(g, feats_dev, wcat, cb):
    KA, KB = g["KA"], g["KB"]

    def b16(a):
        return a.astype(BF16).astype(np.float32)

    x = feats_dev.astype(np.float32)  # [NCORES, NPCP, F] sigma-ordered (bf16 values)
    w32 = np.asarray(wcat, np.float32)
    for l in range(NL):
        y_sh = b16(np.einsum("cnf,fk->cnk", x, w32[l, :, :F]))
        v = b16(np.einsum("cnf,fk->cnk", x, w32[l, :, F:]) + cb[l])
        y_sh[:, :N_PHANTOM, :] = NEG
        table = y_sh.reshape(NTAB, F)
        xn = np.empty_like(x)
        for c in range(NCORES):
            gA = table[g["idxA_flat"][c].astype(np.int64)].reshape(g["CA"], 128, F)
            gB = table[BASE_B + g["idxB_flat"][c].astype(np.int64)].reshape(g["CB"], 128, F)
            for b in range(NB):
                a0, b0 = g["cbA"][b], g["cbB"][b]
                parts = []
                if KA[b] > 0:
                    parts.append(gA[a0 : a0 + KA[b]].max(0))
                if KB[b] > 0:
                    parts.append(gB[b0 : b0 + KB[b]].max(0))
                agg = np.full((128, F), NEG, np.float32) if not parts else (
                    parts[0] if len(parts) == 1 else np.maximum(*parts))
                xn[c, b * 128 : (b + 1) * 128] = np.maximum(
                    agg + v[c, b * 128 : (b + 1) * 128], 0.0)
        x = b16(xn) if l < NL - 1 else xn
    return x


def _make_in_maps(g, feats_dev, wcat, cb2):
    in_maps = []
    for c in range(NCORES):
        in_maps.append({
            "xin": np.ascontiguousarray(feats_dev[c]),
            "idxA": np.ascontiguousarray(g["idxA"][c]),
            "idxB": np.ascontiguousarray(g["idxB"][c]),
            "wcat": wcat,
            "cb2": cb2,
        })
    return in_maps


def _feats_dev(g, feats):
    feats = np.asarray(feats, np.float32)
    fd = np.zeros((NCORES, NPCP, F), BF16)
    core = np.arange(N) // NPC
    fd[core, g["pos"]] = feats.astype(BF16)
    return fd


def _assemble(g, results):
    out_sh = np.stack([np.asarray(r["xout"], np.float32) for r in results])
    core = np.arange(N) // NPC
    return np.ascontiguousarray(out_sh[core, g["pos"]])


def run(feats, src, dst, theta_w, theta_b, phi_w, phi_b, trace=False):
    from concourse.bass_utils import run_bass_kernel_spmd

    key = (src.tobytes()[:64], dst.tobytes()[:64], len(src))
    if _cache.get("graph_key") != key:
        _cache.clear()
        _cache["graph"] = _prep_graph(src, dst)
        _cache["graph_key"] = key
    g = _cache["graph"]
    if "nc" not in _cache:
        _cache["nc"] = _build_kernel(g)
    nc = _cache["nc"]

    wcat, cb2, cb = _prep_weights(theta_w, theta_b, phi_w, phi_b)
    feats_dev = _feats_dev(g, feats)
    in_maps = _make_in_maps(g, feats_dev, wcat, cb2)
    res = run_bass_kernel_spmd(nc, in_maps, core_ids=list(range(NCORES)),
                               trace=trace)
    out = _assemble(g, res.results)
    return out, res


def kernel(feats, src, dst, theta_w, theta_b, phi_w, phi_b):
    out, _ = run(feats, src, dst, theta_w, theta_b, phi_w, phi_b)
    return out


# revision 28
# speedup vs baseline: 1.1040x; 1.0296x over previous
"""EdgeConv GNN (4 layers) on 8 Trainium2 NeuronCores.

Algebraic restructure: with y = x @ theta_w.T and
v = x @ (phi_w - theta_w).T + (phi_b + theta_b),
    msg_e = theta(x[src]-x[dst]) + theta_b + phi(x[dst]) + phi_b
          = y[src] + v[dst]
and since v[dst] is constant within a dst segment:
    out = relu(v + segment_max(y[src], dst))
(nodes with no in-edges come out of segment_max at -1e30 -> relu -> 0,
matching the reference's where(isneginf, 0) + relu).

Distribution: nodes sharded by dst across 8 cores (graph parallel).
Each layer: per-core bf16 matmuls produce its y-shard -> AllGather the
bf16 y table -> SWDGE dma_gather of y rows by src in dst-sorted slot
order -> per-block max-tree reduce on DVE (contiguous tensor_tensor,
not strided tensor_reduce) -> + v -> relu.

dma_gather indices are int16 (<= 32767) so the 50176-row table is
addressed through two windows: A = rows [0, 31360) (src cores 0-4) and
B = rows [18816, 50176) (src cores 3-7). Src cores 3-4 edges are flex
(either window); a per-block joint optimization assigns flex edges to
minimize sum_b(KA[b] + KB[b]) where K* are per-128-node-block degree
caps (shared across the 8 cores: single SPMD instruction stream).
"""

import numpy as np
import ml_dtypes

BF16 = ml_dtypes.bfloat16

N = 50000
NCORES = 8
NPC = 6250            # real nodes per core
NPCP = 6272           # padded nodes per core (49 * 128)
F = 128
NL = 4
NB = NPCP // 128      # 49 blocks per core
NTAB = NCORES * NPCP  # 50176 table rows
BASE_B = 3 * NPCP     # 18816: window B base row (cores 3-7)
N_PHANTOM = NPCP - NPC
GMAX = 40             # max chunks per gather call (per window)
NEG = -1.0e30
SKIP = set()          # debug: subset of {"gather", "reduce", "ag", "mm"}

_cache = {}


# ----------------------------------------------------------------------------
# host-side graph preprocessing
# ----------------------------------------------------------------------------

def _prep_graph(src, dst):
    src = np.asarray(src).astype(np.int64)
    dst = np.asarray(dst).astype(np.int64)
    s_core = src // NPC
    d_core = dst // NPC
    core = np.arange(N) // NPC

    fixedA = s_core <= 2          # rows < 18816: window A only
    fixedB = s_core >= 5          # rows >= 31360: window B only
    flexm = ~fixedA & ~fixedB     # src cores 3-4: rows in both windows
    fA = np.bincount(dst[fixedA], minlength=N)
    fB = np.bincount(dst[fixedB], minlength=N)
    fx = np.bincount(dst[flexm], minlength=N)

    # iterate: (node order by balanced degrees) <-> (per-block joint split)
    kA = np.clip((fB - fA + fx + 1) // 2, 0, fx)
    best = None
    for _ in range(3):
        dA, dB = fA + kA, fB + fx - kA
        pos = np.empty(N, np.int64)
        for c in range(NCORES):
            ids = np.arange(c * NPC, (c + 1) * NPC)
            order = np.lexsort((dA[ids] + dB[ids], np.maximum(dA[ids], dB[ids])))
            pos[ids[order]] = N_PHANTOM + np.arange(NPC)
        blk = pos // 128
        KA = np.zeros(NB, np.int64)
        KB = np.zeros(NB, np.int64)
        kA_new = np.zeros(N, np.int64)
        for b in range(NB):
            m = blk == b
            a, x, bb = fA[m], fx[m], fB[m]
            lo, hi = int(a.max()), int((a + x).max())
            bt = None
            for tA in range(lo, hi + 1):
                kab = np.minimum(x, tA - a)
                tB = int((bb + x - kab).max())
                if bt is None or tA + tB < bt[0] + bt[1]:
                    bt = (tA, tB)
            tA, tB = bt
            kmin = np.maximum(0, x - (tB - bb))
            kmax = np.minimum(x, tA - a)
            kA_new[m] = (kmin + kmax) // 2
            KA[b], KB[b] = tA, tB
        tot = int(KA.sum() + KB.sum())
        if best is None or tot < best[0]:
            best = (tot, KA.copy(), KB.copy(), pos.copy(), kA_new.copy())
        kA = kA_new
    _, KA, KB, pos, kA = best

    sig = core * NPCP + pos       # orig node -> table row
    blk = pos // 128
    lane = pos % 128
    cbA = np.r_[0, np.cumsum(KA)]
    cbB = np.r_[0, np.cumsum(KB)]
    CA, CB = int(cbA[-1]), int(cbB[-1])
    assert KA.max() <= GMAX and KB.max() <= GMAX, (KA.max(), KB.max())

    # edge side: fixed by src core; flex edges ranked within their dst group
    sideA = fixedA.copy()
    fe = np.flatnonzero(flexm)
    fe = fe[np.argsort(dst[fe], kind="stable")]
    dsf = dst[fe]
    starts = np.r_[0, np.flatnonzero(np.diff(dsf)) + 1]
    runlen = np.diff(np.r_[starts, len(dsf)])
    rank = np.arange(len(dsf)) - np.repeat(starts, runlen)
    sideA[fe[rank < kA[dsf]]] = True

    # slot arrays (per core); dummy slots use idx 0 (phantom/NEG row in
    # both windows: row 0 and row BASE_B = core 3 pos 0 are phantoms)
    idxA = np.zeros((NCORES, CA * 128), np.int16)
    idxB = np.zeros((NCORES, CB * 128), np.int16)
    for side, idx_arr, cb, base, cap in (
        (True, idxA, cbA, 0, KA), (False, idxB, cbB, BASE_B, KB)
    ):
        e = np.flatnonzero(sideA == side)
        e = e[np.argsort(dst[e], kind="stable")]
        de = dst[e]
        starts = np.r_[0, np.flatnonzero(np.diff(de)) + 1]
        runlen = np.diff(np.r_[starts, len(de)])
        rank = np.arange(len(de)) - np.repeat(starts, runlen)
        assert (rank < cap[blk[de]]).all()
        slot = (cb[blk[de]] + rank) * 128 + lane[de]
        val = sig[src[e]] - base
        assert val.min() >= 0 and val.max() < 32768, (val.min(), val.max())
        idx_arr[d_core[e], slot] = val.astype(np.int16)

    # wrap indices: [n] -> [128, n//16] int16, replicated across 8 groups of 16
    def wrap(a):
        n = a.shape[1]
        w = a.reshape(NCORES, n // 16, 16).transpose(0, 2, 1)  # [c, 16, n/16]
        return np.ascontiguousarray(
            np.broadcast_to(w[:, None, :, :], (NCORES, 8, 16, n // 16))
        ).reshape(NCORES, 128, n // 16)

    # gather groups: consecutive blocks, chunk budget GMAX per window
    groups = []
    b0 = 0
    while b0 < NB:
        nb = 1
        while (
            b0 + nb < NB
            and cbA[b0 + nb + 1] - cbA[b0] <= GMAX
            and cbB[b0 + nb + 1] - cbB[b0] <= GMAX
        ):
            nb += 1
        groups.append((b0, nb, int(cbA[b0]), int(cbA[b0 + nb] - cbA[b0]),
                       int(cbB[b0]), int(cbB[b0 + nb] - cbB[b0])))
        b0 += nb

    return dict(
        sig=sig, pos=pos, KA=KA, KB=KB, cbA=cbA, cbB=cbB, CA=CA, CB=CB,
        idxA=wrap(idxA), idxB=wrap(idxB), groups=groups,
        idxA_flat=idxA, idxB_flat=idxB,
    )


def _prep_weights(theta_w, theta_b, phi_w, phi_b):
    theta_w = np.asarray(theta_w, np.float32)
    phi_w = np.asarray(phi_w, np.float32)
    cb = (np.asarray(theta_b, np.float32) + np.asarray(phi_b, np.float32))
    wcat = np.concatenate(
        [theta_w.transpose(0, 2, 1), (phi_w - theta_w).transpose(0, 2, 1)], axis=2
    )  # [NL, 128(in), 256(out: y|v)]
    # rank-1 bias row: [NL, 1, 256] = [zeros(128) | cb] — accumulated into the
    # yv PSUM tile via a K=1 matmul with a ones column (frees DVE in mm phase)
    cb2 = np.concatenate([np.zeros((NL, 1, F), np.float32), cb[:, None, :]], axis=2)
    return (np.ascontiguousarray(wcat.astype(BF16)),
            np.ascontiguousarray(cb2.astype(BF16)),
            np.ascontiguousarray(cb))


# ----------------------------------------------------------------------------
# device kernel
# ----------------------------------------------------------------------------

def _build_kernel(g):
    import concourse.bacc as bacc
    import concourse.mybir as mybir
    import concourse.tile as tile
    from concourse.masks import make_identity

    KA, KB, groups = g["KA"], g["KB"], g["groups"]
    cbA, cbB = g["cbA"], g["cbB"]
    CA, CB = g["CA"], g["CB"]

    nc = bacc.Bacc("TRN2", target_bir_lowering=False, debug=False,
                   num_devices=NCORES, num_swdge_queues=4)

    bf16 = mybir.dt.bfloat16
    fp32 = mybir.dt.float32
    Alu = mybir.AluOpType
    Act = mybir.ActivationFunctionType

    xin = nc.dram_tensor("xin", [NPCP, F], bf16, kind="ExternalInput")
    idxA_in = nc.dram_tensor("idxA", [128, CA * 8], mybir.dt.int16, kind="ExternalInput")
    idxB_in = nc.dram_tensor("idxB", [128, CB * 8], mybir.dt.int16, kind="ExternalInput")
    wcat_in = nc.dram_tensor("wcat", [NL, F, 2 * F], bf16, kind="ExternalInput")
    cb2_in = nc.dram_tensor("cb2", [NL, 1, 2 * F], bf16, kind="ExternalInput")
    xout = nc.dram_tensor("xout", [NPCP, F], fp32, kind="ExternalOutput")

    with tile.TileContext(nc) as tc:
        with (
            tc.tile_pool(name="const", bufs=1) as constp,
            tc.tile_pool(name="xp", bufs=2) as xp,
            tc.tile_pool(name="vp", bufs=2) as vp,
            tc.tile_pool(name="wp", bufs=2) as wp,
            tc.tile_pool(name="yp", bufs=3) as yp,
            tc.tile_pool(name="xtp", bufs=3) as xtp,
            tc.tile_pool(name="ga", bufs=3) as gap,
            tc.tile_pool(name="gb", bufs=3) as gbp,
            tc.tile_pool(name="tp", bufs=4) as tp,
            tc.tile_pool(name="ps", bufs=4, space="PSUM") as ps,
            tc.tile_pool(name="dram", bufs=2, space="DRAM") as dram,
        ):
            ident = constp.tile([128, 128], bf16)
            make_identity(nc, ident[:])
            ones = constp.tile([1, 128], bf16)
            nc.vector.memset(ones[:], 1.0)
            zeros = constp.tile([128, F], fp32)
            nc.vector.memset(zeros[:], 0.0)
            qn = [0]  # SWDGE queue round-robin across gather calls
            idxA = constp.tile([128, CA * 8], mybir.dt.int16)
            idxB = constp.tile([128, CB * 8], mybir.dt.int16)
            nc.sync.dma_start(idxA[:], idxA_in[:])
            nc.sync.dma_start(idxB[:], idxB_in[:])

            x = xp.tile([128, NB, F], bf16, tag="x")
            nc.sync.dma_start(x[:], xin.rearrange("(b p) f -> p b f", p=128))

            for l in range(NL):
                W = wp.tile([128, 2 * F], bf16, tag="w")
                nc.sync.dma_start(W[:], wcat_in[l])
                cb2 = wp.tile([1, 2 * F], bf16, tag="cb2")
                nc.sync.dma_start(cb2[:], cb2_in[l])

                y_ag_in = dram.tile([NPCP, F], bf16, tag="yag")
                y_all = dram.tile([NTAB, F], bf16, tag="yall", addr_space="Shared")

                v = vp.tile([128, NB, F], bf16, tag="v")

                # ---- matmul phase: y (table, bf16) and v; DVE-free so the
                # vector engine stays on the (previous layer's) reduce work
                for t in range(NB):
                    xT_ps = ps.tile([128, 128], bf16, tag="xt_ps")
                    nc.tensor.transpose(xT_ps[:], x[:, t, :], ident[:])
                    xT = xtp.tile([128, 128], bf16, tag="xt")
                    nc.scalar.activation(xT[:], xT_ps[:], Act.Copy)
                    yv_ps = ps.tile([128, 2 * F], fp32, tag="yv_ps")
                    nc.tensor.matmul(yv_ps[:], lhsT=xT[:], rhs=W[:],
                                     start=True, stop=False)
                    nc.tensor.matmul(yv_ps[:], lhsT=ones[:], rhs=cb2[:],
                                     start=False, stop=True)
                    y_sb = yp.tile([128, F], bf16, tag="y")
                    nc.scalar.activation(y_sb[:], yv_ps[:, 0:F], Act.Copy)
                    if t == 0:
                        nc.vector.memset(y_sb[0:N_PHANTOM, :], NEG)
                    nc.sync.dma_start(y_ag_in[t * 128 : (t + 1) * 128, :], y_sb[:])
                    nc.scalar.activation(v[:, t, :], yv_ps[:, F : 2 * F], Act.Copy)

                if "ag" not in SKIP:
                    nc.gpsimd.collective_compute(
                        "AllGather",
                        Alu.bypass,
                        replica_groups=[list(range(NCORES))],
                        ins=[y_ag_in.opt()],
                        outs=[y_all.opt()],
                    )
                else:
                    nc.sync.dma_start(y_all[0:NPCP, :], y_ag_in[:])

                # ---- gather + segment-max phase ----
                if l < NL - 1:
                    x_next = xp.tile([128, NB, F], bf16, tag="x")
                for (b0, nbl, aoff, acnt, boff, bcnt) in groups:
                    gA = gap.tile([128, GMAX, F], bf16, tag="ga")
                    gB = gbp.tile([128, GMAX, F], bf16, tag="gb")
                    if "gather" in SKIP:
                        nc.vector.memset(gA[:], 0.0)
                        nc.vector.memset(gB[:], 0.0)
                    else:
                        # SWDGE ring fits only ~8 chunks per call: 65
                        # descs/lane works, 97+ (12-chunk) hangs the device.
                        # A/B calls interleaved so the 4 SWDGE queues always
                        # have independent work adjacent in the dispatch order
                        ca = [("A", o, min(8, acnt - o)) for o in range(0, acnt, 8)]
                        cb_ = [("B", o, min(8, bcnt - o)) for o in range(0, bcnt, 8)]
                        inter = []
                        for i in range(max(len(ca), len(cb_))):
                            if i < len(ca):
                                inter.append(ca[i])
                            if i < len(cb_):
                                inter.append(cb_[i])
                        for side, o, n in inter:
                            if side == "A":
                                nc.gpsimd.dma_gather(
                                    gA[:, o : o + n, :], y_all[:, :],
                                    idxA[:, (aoff + o) * 8 : (aoff + o + n) * 8],
                                    n * 128, n * 128, F,
                                    queue_num=qn[0] % 4,
                                )
                            else:
                                nc.gpsimd.dma_gather(
                                    gB[:, o : o + n, :], y_all[BASE_B:, :],
                                    idxB[:, (boff + o) * 8 : (boff + o + n) * 8],
                                    n * 128, n * 128, F,
                                    queue_num=qn[0] % 4,
                                )
                            qn[0] += 1
                    for b in range(b0, b0 + nbl):
                        ka = int(cbA[b] - cbA[b0])
                        kb = int(cbB[b] - cbB[b0])
                        ha, hb = int(KA[b]), int(KB[b])
                        # in-place max tree per window over contiguous slices
                        for gt, o, c in ((gA, ka, ha), (gB, kb, hb)):
                            while c > 1:
                                h = c // 2
                                nc.vector.tensor_tensor(
                                    out=gt[:, o : o + h, :],
                                    in0=gt[:, o : o + h, :],
                                    in1=gt[:, o + c - h : o + c, :],
                                    op=Alu.max)
                                c -= h
                        tv = tp.tile([128, F], fp32, tag="tv")
                        if ha > 0 and hb > 0:
                            tM = tp.tile([128, F], bf16, tag="tm")
                            nc.vector.tensor_tensor(out=tM[:], in0=gA[:, ka, :],
                                                    in1=gB[:, kb, :], op=Alu.max)
                            nc.vector.tensor_tensor(out=tv[:], in0=tM[:],
                                                    in1=v[:, b, :], op=Alu.add)
                        elif ha > 0 or hb > 0:
                            srcg = gA[:, ka, :] if ha > 0 else gB[:, kb, :]
                            nc.vector.tensor_tensor(out=tv[:], in0=srcg,
                                                    in1=v[:, b, :], op=Alu.add)
                        else:
                            nc.vector.memset(tv[:], NEG)
                        # relu on DVE: keeps the scalar engine free so the
                        # next layer's matmul-phase copies overlap this phase
                        # (tensor_tensor vs a zeros tile — tensor_scalar with
                        # a float const takes a 5x slower DVE path)
                        if l < NL - 1:
                            nc.vector.tensor_tensor(out=x_next[:, b, :], in0=tv[:],
                                                    in1=zeros[:], op=Alu.max)
                        else:
                            xo = tp.tile([128, F], fp32, tag="xo")
                            nc.vector.tensor_tensor(out=xo[:], in0=tv[:],
                                                    in1=zeros[:], op=Alu.max)
                            nc.sync.dma_start(
                                xout[b * 128 : (b + 1) * 128, :], xo[:])
                if l < NL - 1:
                    x = x_next

    nc.compile()
    return nc


# ----------------------------------------------------------------------------
# numpy emulation of the device dataflow (for validating prep structures)
# ----------------------------------------------------------------------------

def _emulate(g, feats_dev, wcat, cb):
    KA, KB = g["KA"], g["KB"]

    def b16(a):
        return a.astype(BF16).astype(np.float32)

    x = feats_dev.astype(np.float32)  # [NCORES, NPCP, F] sigma-ordered (bf16 values)
    w32 = np.asarray(wcat, np.float32)
    for l in range(NL):
        y_sh = b16(np.einsum("cnf,fk->cnk", x, w32[l, :, :F]))
        v = b16(np.einsum("cnf,fk->cnk", x, w32[l, :, F:]) + cb[l])
        y_sh[:, :N_PHANTOM, :] = NEG
        table = y_sh.reshape(NTAB, F)
        xn = np.empty_like(x)
        for c in range(NCORES):
            gA = table[g["idxA_flat"][c].astype(np.int64)].reshape(g["CA"], 128, F)
            gB = table[BASE_B + g["idxB_flat"][c].astype(np.int64)].reshape(g["CB"], 128, F)
            for b in range(NB):
                a0, b0 = g["cbA"][b], g["cbB"][b]
                parts = []
                if KA[b] > 0:
                    parts.append(gA[a0 : a0 + KA[b]].max(0))
                if KB[b] > 0:
                    parts.append(gB[b0 : b0 + KB[b]].max(0))
                agg = np.full((128, F), NEG, np.float32) if not parts else (
                    parts[0] if len(parts) == 1 else np.maximum(*parts))
                xn[c, b * 128 : (b + 1) * 128] = np.maximum(
                    agg + v[c, b * 128 : (b + 1) * 128], 0.0)
        x = b16(xn) if l < NL - 1 else xn
    return x


def _make_in_maps(g, feats_dev, wcat, cb2):
    in_maps = []
    for c in range(NCORES):
        in_maps.append({
            "xin": np.ascontiguousarray(feats_dev[c]),
            "idxA": np.ascontiguousarray(g["idxA"][c]),
            "idxB": np.ascontiguousarray(g["idxB"][c]),
            "wcat": wcat,
            "cb2": cb2,
        })
    return in_maps


def _feats_dev(g, feats):
    feats = np.asarray(feats, np.float32)
    fd = np.zeros((NCORES, NPCP, F), BF16)
    core = np.arange(N) // NPC
    fd[core, g["pos"]] = feats.astype(BF16)
    return fd


def _assemble(g, results):
    out_sh = np.stack([np.asarray(r["xout"], np.float32) for r in results])
    core = np.arange(N) // NPC
    return np.ascontiguousarray(out_sh[core, g["pos"]])


def run(feats, src, dst, theta_w, theta_b, phi_w, phi_b, trace=False):
    from concourse.bass_utils import run_bass_kernel_spmd

    key = (src.tobytes()[:64], dst.tobytes()[:64], len(src))
    if _cache.get("graph_key") != key:
        _cache.clear()
        _cache["graph"] = _prep_graph(src, dst)
        _cache["graph_key"] = key
    g = _cache["graph"]
    if "nc" not in _cache:
        _cache["nc"] = _build_kernel(g)
    nc = _cache["nc"]

    wcat, cb2, cb = _prep_weights(theta_w, theta_b, phi_w, phi_b)
    feats_dev = _feats_dev(g, feats)
    in_maps = _make_in_maps(g, feats_dev, wcat, cb2)
    res = run_bass_kernel_spmd(nc, in_maps, core_ids=list(range(NCORES)),
                               trace=trace)
    out = _assemble(g, res.results)
    return out, res


def kernel(feats, src, dst, theta_w, theta_b, phi_w, phi_b):
    out, _ = run(feats, src, dst, theta_w, theta_b, phi_w, phi_b)
    return out
